# revision 1
# baseline (speedup 1.0000x reference)
"""BigBird block-sparse attention kernel for 8 Trainium2 NeuronCores.

Sharding: data-parallel over batch (B=2) x head-parallel over head groups
(16 heads -> 4 groups of 4). Core c handles batch c//4, heads [4*(c%4), 4*(c%4)+4).
Each core computes its Q/K/V projection column slice, block-sparse attention for
its 4 heads (processed as 2 "pairs" of 2 heads packed on 128 partitions), and a
partial output projection. Host sums the 4 partials per batch and adds bo.

The attention is computed in transposed score layout (scoresT[kv, q]) so the
probability tiles feed the PV matmul directly -- no on-chip transposes or
gathers are needed; the middle-row units are DMA-free. Row sums for the softmax
come from a ones-matrix matmul in the same array mode; normalization happens in
the PSUM->SBUF epilogue.

Self-contained: hardcodes shapes; derives the block-sparsity structure from the
block_mask input at trace time.
"""

import os
import numpy as np
import ml_dtypes

import concourse.bass as bass
import concourse.mybir as mybir
import concourse.tile as tile
from concourse import bacc
from concourse.bass_utils import run_bass_kernel_spmd

F32 = mybir.dt.float32
BF16 = mybir.dt.bfloat16

B, S, E, H = 2, 2048, 1024, 16
BS = 64                      # block size
NB = S // BS                 # 32 blocks
HD = E // H                  # 64 head dim
SCALE = HD ** -0.5           # 0.125
NCORES = 8
GROUPS = 4                   # head groups (one per core within a batch)
COLS = E // GROUPS           # 256 projection cols per core
PAIRS = 2                    # head pairs per core (2 heads = 128 cols each)

LAST_RESULTS = None          # BassKernelResults of the last run (for test.py)

DEFAULT_CFG = dict(
    ps_proj=4, o_bufs=4,
    sc_m=2, pv_m=2, sm_m=2, pt_m=3,
    epi_engine="vector",
)


def _build_program(sel_lists, cfg=None):
    """Build the SPMD bass program. sel_lists[i] = sorted kv block list of q block i."""
    cfg = dict(DEFAULT_CFG, **(cfg or {}))
    nc = bacc.Bacc("TRN2", target_bir_lowering=False, debug=False)

    xT_d = nc.dram_tensor("xT", [E, S], BF16, kind="ExternalInput")
    wq_d = nc.dram_tensor("wq", [E, COLS], BF16, kind="ExternalInput")
    wk_d = nc.dram_tensor("wk", [E, COLS], BF16, kind="ExternalInput")
    wv_d = nc.dram_tensor("wv", [E, COLS], BF16, kind="ExternalInput")
    wo_d = nc.dram_tensor("wo", [COLS, E], BF16, kind="ExternalInput")
    bq_d = nc.dram_tensor("bq", [COLS], F32, kind="ExternalInput")
    bk_d = nc.dram_tensor("bk", [COLS], F32, kind="ExternalInput")
    bv_d = nc.dram_tensor("bv", [COLS], F32, kind="ExternalInput")
    out_d = nc.dram_tensor("out", [S, E], BF16, kind="ExternalOutput")

    EC = E // 128              # 8 contraction chunks
    ST = 512                   # S tile for projections
    NST = S // ST              # 4

    glob_rows = [i for i in range(NB) if len(sel_lists[i]) == NB]
    mid_rows = [i for i in range(NB) if len(sel_lists[i]) != NB]
    for i in mid_rows:
        assert len(sel_lists[i]) == 6, (i, len(sel_lists[i]))

    with tile.TileContext(nc) as tc:
        with (
            tc.tile_pool(name="persist", bufs=1) as persist,
        ):
            # ---- persistent SBUF tensors ----
            xT_s = persist.tile([128, EC, S], BF16, tag="xT_s")
            wq_s = persist.tile([128, EC, COLS], BF16, tag="wq_s")
            wk_s = persist.tile([128, EC, COLS], BF16, tag="wk_s")
            wv_s = persist.tile([128, EC, COLS], BF16, tag="wv_s")
            wo_s = persist.tile([128, PAIRS, 2, 512], BF16, tag="wo_s")
            bq_s = persist.tile([128, PAIRS], F32, tag="bq_s")
            bk_s = persist.tile([128, PAIRS], F32, tag="bk_s")
            bv_s = persist.tile([128, PAIRS], F32, tag="bv_s")
            ones_m = persist.tile([128, 128], BF16, tag="ones_m")
            qdiag = [persist.tile([128, NB, 128], BF16, tag=f"qdiag{p}",
                                  name=f"qdiag{p}") for p in range(PAIRS)]
            kT = [persist.tile([128, S], BF16, tag=f"kT{p}", name=f"kT{p}")
                  for p in range(PAIRS)]
            # vp: kv-chunk layout [kv%128, chunk, (2h,HD)] (global-row PV)
            vp = [persist.tile([128, NB // 2, 128], BF16, tag=f"vp{p}",
                               name=f"vp{p}") for p in range(PAIRS)]
            # vdup: per-block layout duplicated on both partition halves
            vdup = [persist.tile([128, NB, 128], BF16, tag=f"vdup{p}",
                                 name=f"vdup{p}") for p in range(PAIRS)]
            attn = [persist.tile([128, NB, BS], BF16, tag=f"attn{p}",
                                 name=f"attn{p}") for p in range(PAIRS)]

            # ---- input loads (wq + first xT tile first, so PE starts ASAP) ----
            nc.sync.dma_start(wq_s[:], wq_d.ap().rearrange("(c p) m -> p c m", p=128))
            xT_view = xT_d.ap().rearrange("(c p) s -> p c s", p=128)
            nc.sync.dma_start(xT_s[:, :, 0:ST], xT_view[:, :, 0:ST])
            nc.scalar.dma_start(bq_s[:], bq_d.ap().rearrange("(pp p) -> p pp", p=128))
            nc.scalar.dma_start(bk_s[:], bk_d.ap().rearrange("(pp p) -> p pp", p=128))
            nc.scalar.dma_start(bv_s[:], bv_d.ap().rearrange("(pp p) -> p pp", p=128))
            nc.sync.dma_start(wk_s[:], wk_d.ap().rearrange("(c p) m -> p c m", p=128))
            nc.sync.dma_start(wv_s[:], wv_d.ap().rearrange("(c p) m -> p c m", p=128))
            for t in range(1, NST):
                sl = slice(t * ST, (t + 1) * ST)
                nc.sync.dma_start(xT_s[:, :, sl], xT_view[:, :, sl])
            # wo is consumed only by the (late) output projection
            nc.sync.dma_start(
                wo_s[:],
                wo_d.ap().rearrange("(pp p) (h f) -> p pp h f", p=128, f=512),
            )

            nc.gpsimd.memset(ones_m[:], 1.0)
            for p in range(PAIRS):
                nc.gpsimd.memset(qdiag[p][:], 0.0)

            # ---- phase 1: Q/K/V projections (+ global rows per pair) ----
            def glob_unit(p, i, sc_g, pv_g, sm_g, pt_g, ep_g):
                NCH = NB // 2  # 16 kv chunks, in two 8-chunk halves
                ps_pv = pv_g.tile([128, 128], F32, tag="pv", name="pv")
                ps_sm = sm_g.tile([128, 128], F32, tag="sm", name="sm")
                for half in range(2):
                    c0 = half * (NCH // 2)
                    ps_s = sc_g.tile([128, NCH // 2, 128], F32, tag="sc", name="sc")
                    PT = pt_g.tile([128, NCH // 2, 128], BF16, tag="pt", name="pt")
                    for cc in range(NCH // 2):
                        nc.tensor.matmul(
                            ps_s[:, cc, :],
                            kT[p][:, (c0 + cc) * 128:(c0 + cc + 1) * 128],
                            qdiag[p][:, i, :],
                            start=True, stop=True,
                        )
                    for c4 in range(2):   # one ACT per PSUM bank
                        sl = slice(c4 * 4, c4 * 4 + 4)
                        nc.scalar.activation(
                            PT[:, sl, :], ps_s[:, sl, :],
                            mybir.ActivationFunctionType.Exp, scale=SCALE,
                        )
                    for cc in range(NCH // 2):
                        nc.tensor.matmul(
                            ps_pv[:], vp[p][:, c0 + cc, :], PT[:, cc, :],
                            start=(half == 0 and cc == 0),
                            stop=(half == 1 and cc == NCH // 2 - 1),
                        )
                    for cc in range(NCH // 2):
                        nc.tensor.matmul(
                            ps_sm[:], ones_m[:], PT[:, cc, :],
                            start=(half == 0 and cc == 0),
                            stop=(half == 1 and cc == NCH // 2 - 1),
                        )
                rc = ep_g.tile([128, 128], F32, tag="rc", name="rc")
                nc.vector.reciprocal(rc[:], ps_sm[:])
                nc.vector.tensor_tensor(
                    attn[p][0:64, i, :], ps_pv[0:64, 0:64],
                    rc[0:64, 0:64], mybir.AluOpType.mult)
                nc.vector.tensor_tensor(
                    attn[p][64:128, i, :], ps_pv[64:128, 64:128],
                    rc[64:128, 64:128], mybir.AluOpType.mult)

            if "proj" in cfg.get("phases", ("proj", "glob", "mid", "out")):
             with (
                tc.tile_pool(name="ps_proj", bufs=cfg["ps_proj"], space="PSUM") as ps_proj,
                tc.tile_pool(name="vt_tmp", bufs=3) as vt_pool,
                tc.tile_pool(name="sc_g", bufs=1, space="PSUM") as sc_g,
                tc.tile_pool(name="pv_g", bufs=1, space="PSUM") as pv_g,
                tc.tile_pool(name="sm_g", bufs=1, space="PSUM") as sm_g,
                tc.tile_pool(name="pt_g", bufs=2) as pt_g,
                tc.tile_pool(name="ep_g", bufs=2) as ep_g,
            ):
                do_glob = "glob" in cfg.get("phases", ("proj", "glob", "mid", "out"))
                for p in range(PAIRS):
                    pcol = slice(p * 128, (p + 1) * 128)
                    for t in range(NST):
                        ssl = slice(t * ST, (t + 1) * ST)
                        # Q -> qT -> qdiag (block-diagonal per q block)
                        ps = ps_proj.tile([128, ST], F32, tag="ps")
                        for c in range(EC):
                            nc.tensor.matmul(
                                ps[:], wq_s[:, c, pcol], xT_s[:, c, ssl],
                                start=(c == 0), stop=(c == EC - 1),
                            )
                        nblk = ST // BS  # 8 blocks per S tile
                        b0 = t * nblk
                        src = ps.rearrange("q (nb f) -> q nb f", f=BS)
                        nc.vector.tensor_scalar(
                            qdiag[p][0:64, b0:b0 + nblk, 0:64],
                            src[0:64], bq_s[0:64, p:p + 1], None,
                            mybir.AluOpType.add,
                        )
                        nc.vector.tensor_scalar(
                            qdiag[p][64:128, b0:b0 + nblk, 64:128],
                            src[64:128], bq_s[64:128, p:p + 1], None,
                            mybir.AluOpType.add,
                        )
                        # K -> kT
                        ps = ps_proj.tile([128, ST], F32, tag="ps")
                        for c in range(EC):
                            nc.tensor.matmul(
                                ps[:], wk_s[:, c, pcol], xT_s[:, c, ssl],
                                start=(c == 0), stop=(c == EC - 1),
                            )
                        nc.scalar.activation(
                            kT[p][:, ssl], ps[:],
                            mybir.ActivationFunctionType.Identity,
                            bias=bk_s[:, p:p + 1],
                        )
                        # V -> vT tmp -> DMA-transpose -> vp chunks
                        ps = ps_proj.tile([128, ST], F32, tag="ps")
                        for c in range(EC):
                            nc.tensor.matmul(
                                ps[:], wv_s[:, c, pcol], xT_s[:, c, ssl],
                                start=(c == 0), stop=(c == EC - 1),
                            )
                        vt = vt_pool.tile([128, ST], BF16, tag="vt")
                        nc.vector.tensor_scalar(
                            vt[:], ps[:], bv_s[:, p:p + 1], None,
                            mybir.AluOpType.add,
                        )
                        for j in range(ST // 128):
                            nc.scalar.dma_start(
                                vp[p][:, t * (ST // 128) + j, :],
                                vt[:, j * 128:(j + 1) * 128],
                                transpose=True,
                            )
                    # pair p projections done: build vdup, emit global rows
                    for half in range(2):
                        hs = slice(half * 64, half * 64 + 64)
                        nc.sync.dma_start(vdup[p][hs, 0:NB:2, :], vp[p][0:64, :, :])
                        nc.sync.dma_start(vdup[p][hs, 1:NB:2, :], vp[p][64:128, :, :])
                    if do_glob:
                        for i in glob_rows:
                            glob_unit(p, i, sc_g, pv_g, sm_g, pt_g, ep_g)



            # (global rows are emitted inside the projection scope, per pair)

            # ---- phase 3: middle rows (6 kv blocks each), DMA-free ----
            # Units run in couples sharing one scoresT PSUM tile / PT tile:
            # couple partition halves hold the two units.
            if "mid" in cfg.get("phases", ("proj", "glob", "mid", "out")):
             out_view = out_d.ap().rearrange("(t p) (h f) -> t p h f", p=128, f=512)
             done_rows = set(glob_rows)
             emitted_t = set()
             with (
                tc.tile_pool(name="sc_m", bufs=cfg["sc_m"], space="PSUM") as sc_m,
                tc.tile_pool(name="pv_m", bufs=cfg["pv_m"], space="PSUM") as pv_m,
                tc.tile_pool(name="sm_m", bufs=1, space="PSUM") as sm_m,
                tc.tile_pool(name="ps_oi", bufs=1, space="PSUM") as ps_oi,
                tc.tile_pool(name="pt_m", bufs=cfg["pt_m"]) as pt_m,
                tc.tile_pool(name="ep_m", bufs=4) as ep_m,
                tc.tile_pool(name="o_tmp", bufs=4) as o_pool,
            ):
                couples = []
                if cfg.get("couple_order", "interleave") == "interleave":
                    for k in range(0, len(mid_rows), 2):
                        for p in range(PAIRS):
                            couples.append((p, mid_rows[k:k + 2]))
                else:
                    for p in range(PAIRS):
                        for k in range(0, len(mid_rows), 2):
                            couples.append((p, mid_rows[k:k + 2]))
                for p, rows in couples:
                    ps_s = sc_m.tile([128, 6, 128], F32, tag="sc", name="sc")
                    PT = pt_m.tile([128, 6, 128], BF16, tag="pt", name="pt")
                    for u, i in enumerate(rows):
                        hs = slice(u * 64, u * 64 + 64)
                        for j, b in enumerate(sel_lists[i]):
                            nc.tensor.matmul(
                                ps_s[hs, j, :],
                                kT[p][:, b * BS:(b + 1) * BS],
                                qdiag[p][:, i, :],
                                start=True, stop=True,
                            )
                    # exp per PSUM bank: slots 0-3 (bank0), 4-5 (bank1)
                    nc.scalar.activation(
                        PT[:, 0:4, :], ps_s[:, 0:4, :],
                        mybir.ActivationFunctionType.Exp, scale=SCALE)
                    nc.scalar.activation(
                        PT[:, 4:6, :], ps_s[:, 4:6, :],
                        mybir.ActivationFunctionType.Exp, scale=SCALE)
                    for u, i in enumerate(rows):
                        hs = slice(u * 64, u * 64 + 64)
                        ps_pv = pv_m.tile([128, 128], F32, tag="pv", name="pv")
                        ps_sm = sm_m.tile([128, 128], F32, tag="sm", name="sm")
                        for j, b in enumerate(sel_lists[i]):
                            nc.tensor.matmul(
                                ps_pv[:], vdup[p][hs, b, :], PT[hs, j, :],
                                start=(j == 0), stop=(j == 5),
                            )
                        for j in range(6):
                            nc.tensor.matmul(
                                ps_sm[:], ones_m[hs, :], PT[hs, j, :],
                                start=(j == 0), stop=(j == 5),
                            )
                        rc = ep_m.tile([128, 128], F32, tag="rc", name="rc")
                        nc.vector.reciprocal(rc[:], ps_sm[:])
                        nc.vector.tensor_tensor(
                            attn[p][0:64, i, :], ps_pv[0:64, 0:64],
                            rc[0:64, 0:64], mybir.AluOpType.mult)
                        nc.vector.tensor_tensor(
                            attn[p][64:128, i, :], ps_pv[64:128, 64:128],
                            rc[64:128, 64:128], mybir.AluOpType.mult)
                        done_rows.add(i)
                    # emit output-projection tiles whose attn inputs are ready
                    for t in range(S // 128):
                        if t in emitted_t:
                            continue
                        if 2 * t in done_rows and 2 * t + 1 in done_rows and \
                                p == PAIRS - 1:
                            emitted_t.add(t)
                            for h in range(2):
                                pso = ps_oi.tile([128, 512], F32, tag="po", name="po")
                                for pp in range(PAIRS):
                                    nc.tensor.matmul(
                                        pso[:],
                                        attn[pp][:, 2 * t:2 * t + 2, :],
                                        wo_s[:, pp, h, :],
                                        start=(pp == 0), stop=(pp == PAIRS - 1),
                                    )
                                ot = o_pool.tile([128, 512], BF16, tag="ot")
                                if (t + h) % 2 == 0:
                                    nc.scalar.copy(ot[:], pso[:])
                                    nc.sync.dma_start(out_view[t, :, h, :], ot[:])
                                else:
                                    nc.vector.tensor_copy(ot[:], pso[:])
                                    nc.scalar.dma_start(out_view[t, :, h, :], ot[:])

            # ---- phase 4: output projection (partial over this core's cols) ----
            out_view = out_d.ap().rearrange("(t p) (h f) -> t p h f", p=128, f=512)
            if "out" in cfg.get("phases", ("proj", "glob", "mid", "out")):
             with (
                tc.tile_pool(name="ps_out", bufs=cfg["ps_proj"], space="PSUM") as ps_out,
                tc.tile_pool(name="o_tmp", bufs=cfg["o_bufs"]) as o_pool,
            ):
                for t in range(S // 128):
                    if t in emitted_t:
                        continue
                    for h in range(2):
                        ps = ps_out.tile([128, 512], F32, tag="po")
                        for p in range(PAIRS):
                            nc.tensor.matmul(
                                ps[:],
                                attn[p][:, 2 * t:2 * t + 2, :],
                                wo_s[:, p, h, :],
                                start=(p == 0), stop=(p == PAIRS - 1),
                            )
                        ot = o_pool.tile([128, 512], BF16, tag="ot")
                        if (t + h) % 2 == 0:
                            nc.scalar.copy(ot[:], ps[:])
                        else:
                            nc.vector.tensor_copy(ot[:], ps[:])
                        nc.sync.dma_start(out_view[t, :, h, :], ot[:])

    nc.compile()
    return nc


_cache = {}


def _get_program(block_mask, cfg=None):
    bm = np.asarray(block_mask)
    assert bm.shape == (S, S)
    blk = bm.reshape(NB, BS, NB, BS).any(axis=(1, 3))
    key = (blk.tobytes(), tuple(sorted((cfg or {}).items())))
    if key not in _cache:
        sel_lists = [list(np.nonzero(blk[i])[0]) for i in range(NB)]
        _cache[key] = (_build_program(sel_lists, cfg), sel_lists)
    return _cache[key]


def kernel(x, Wq, bq, Wk, bk, Wv, bv, Wo, bo, block_mask):
    global LAST_RESULTS
    x = np.asarray(x)
    nc, _ = _get_program(block_mask)

    bf = ml_dtypes.bfloat16
    in_maps = []
    for c in range(NCORES):
        b = c // GROUPS
        g = c % GROUPS
        cols = slice(g * COLS, (g + 1) * COLS)
        in_maps.append({
            "xT": np.ascontiguousarray(np.asarray(x)[b].T).astype(bf),
            "wq": np.ascontiguousarray(np.asarray(Wq)[:, cols]).astype(bf),
            "wk": np.ascontiguousarray(np.asarray(Wk)[:, cols]).astype(bf),
            "wv": np.ascontiguousarray(np.asarray(Wv)[:, cols]).astype(bf),
            "wo": np.ascontiguousarray(np.asarray(Wo)[cols, :]).astype(bf),
            "bq": np.ascontiguousarray(np.asarray(bq)[cols]).astype(np.float32),
            "bk": np.ascontiguousarray(np.asarray(bk)[cols]).astype(np.float32),
            "bv": np.ascontiguousarray(np.asarray(bv)[cols]).astype(np.float32),
        })

    trace = bool(int(os.environ.get("KERNEL_TRACE", "0")))
    try:
        res = run_bass_kernel_spmd(
            nc, in_maps, core_ids=list(range(NCORES)), trace=trace,
        )
    except ModuleNotFoundError:
        # axon NTFF profile hook not available in this container
        res = run_bass_kernel_spmd(
            nc, in_maps, core_ids=list(range(NCORES)), trace=False,
        )
    LAST_RESULTS = res

    out = np.zeros((B, S, E), dtype=np.float32)
    for c in range(NCORES):
        out[c // GROUPS] += res.results[c]["out"].astype(np.float32)
    out += np.asarray(bo, dtype=np.float32)
    return out



# revision 9
# speedup vs baseline: 1.1511x; 1.1511x over previous
"""BigBird block-sparse attention kernel for 8 Trainium2 NeuronCores.

Sharding: data-parallel over batch (B=2) x head-parallel over head groups
(16 heads -> 4 groups of 4). Core c handles batch c//4, heads [4*(c%4), 4*(c%4)+4).
Each core computes its Q/K/V projection column slice, block-sparse attention for
its 4 heads (processed as 2 "pairs" of 2 heads packed on 128 partitions), and a
partial output projection. Host sums the 4 partials per batch and adds bo.

Attention uses transposed score layout (scoresT[kv, q]) so probability tiles
feed the PV matmul directly. Middle rows pack their 6 kv blocks into 3
score-PSUM slots of 128 kv each; consecutive block pairs use a single
128-contraction matmul (kT columns are contiguous), arbitrary pairs fall back
to two 64-partition matmuls per slot. Softmax row sums always contract 128 kv
per matmul (3 ones-matmuls per row). PV uses vp (kv%128-major V) plus vshift
(V shifted by one block) so consecutive pairs contract 128 kv in one matmul.

Self-contained: hardcodes shapes; derives the block-sparsity structure from the
block_mask input at trace time.
"""

import os
import numpy as np
import ml_dtypes

import concourse.bass as bass
import concourse.mybir as mybir
import concourse.tile as tile
from concourse import bacc
from concourse.bass_utils import run_bass_kernel_spmd

F32 = mybir.dt.float32
BF16 = mybir.dt.bfloat16

B, S, E, H = 2, 2048, 1024, 16
BS = 64                      # block size
NB = S // BS                 # 32 blocks
HD = E // H                  # 64 head dim
SCALE = HD ** -0.5           # 0.125
NCORES = 8
GROUPS = 4                   # head groups (one per core within a batch)
COLS = E // GROUPS           # 256 projection cols per core
PAIRS = 2                    # head pairs per core (2 heads = 128 cols each)
EC = E // 128                # 8 contraction chunks
ST = 512                     # S tile for projections
NST = S // ST                # 4

LAST_RESULTS = None          # BassKernelResults of the last run (for test.py)

DEFAULT_CFG = dict(
    ps_proj=3, sc_m=2, pv_m=2, sm_m=2, pt_m=3, rc_m=2,
    copy_acts=True,          # alternate outproj copies between ACT and DVE
)


def _env_flag(name, default="0"):
    return bool(int(os.environ.get(name, default)))


def _make_pairs(sel):
    """Pair the 6 sorted kv blocks of a middle row into 3 slots.

    Returns list of ('c', a) for a consecutive pair (a, a+1) or
    ('s', a, b) for an arbitrary pair. Greedy left-to-right consecutive
    matching; leftovers paired in order (a < b so b is never block 0).
    """
    used = [False] * len(sel)
    pairs = []
    if _env_flag("KERNEL_ALL_SPLIT"):
        return [('s', sel[0], sel[1]), ('s', sel[2], sel[3]), ('s', sel[4], sel[5])]
    i = 0
    while i < len(sel) - 1:
        if not used[i] and not used[i + 1] and sel[i + 1] == sel[i] + 1:
            pairs.append(('c', sel[i]))
            used[i] = used[i + 1] = True
            i += 2
        else:
            i += 1
    rest = [sel[i] for i in range(len(sel)) if not used[i]]
    for j in range(0, len(rest), 2):
        pairs.append(('s', rest[j], rest[j + 1]))
    assert len(pairs) == 3
    return pairs


def _build_program(sel_lists, cfg=None):
    """Build the SPMD bass program. sel_lists[i] = sorted kv block list of q block i."""
    cfg = dict(DEFAULT_CFG, **(cfg or {}))
    nc = bacc.Bacc("TRN2", target_bir_lowering=False, debug=False)

    xT_d = nc.dram_tensor("xT", [E, S], BF16, kind="ExternalInput")
    wq_d = nc.dram_tensor("wq", [E, COLS], BF16, kind="ExternalInput")
    wk_d = nc.dram_tensor("wk", [E, COLS], BF16, kind="ExternalInput")
    wv_d = nc.dram_tensor("wv", [E, COLS], BF16, kind="ExternalInput")
    wo_d = nc.dram_tensor("wo", [COLS, E], BF16, kind="ExternalInput")
    bq_d = nc.dram_tensor("bq", [COLS], F32, kind="ExternalInput")
    bk_d = nc.dram_tensor("bk", [COLS], F32, kind="ExternalInput")
    bv_d = nc.dram_tensor("bv", [COLS], F32, kind="ExternalInput")
    out_d = nc.dram_tensor("out", [S, E], BF16, kind="ExternalOutput")

    glob_rows = [i for i in range(NB) if len(sel_lists[i]) == NB]
    mid_rows = [i for i in range(NB) if len(sel_lists[i]) != NB]
    for i in mid_rows:
        assert len(sel_lists[i]) == 6, (i, len(sel_lists[i]))
    mid_pairs = {i: _make_pairs(sel_lists[i]) for i in mid_rows}

    with tile.TileContext(nc) as tc:
        with (
            tc.tile_pool(name="persist", bufs=1) as persist,
        ):
            # ---- persistent SBUF tensors ----
            xT_s = persist.tile([128, EC, S], BF16, tag="xT_s")
            wq_s = persist.tile([128, EC, COLS], BF16, tag="wq_s")
            wk_s = persist.tile([128, EC, COLS], BF16, tag="wk_s")
            wv_s = persist.tile([128, EC, COLS], BF16, tag="wv_s")
            wo_s = persist.tile([128, PAIRS, 2, 512], BF16, tag="wo_s")
            bq_s = persist.tile([128, PAIRS], F32, tag="bq_s")
            bk_s = persist.tile([128, PAIRS], F32, tag="bk_s")
            bv_s = persist.tile([128, PAIRS], F32, tag="bv_s")
            ones_m = persist.tile([128, 128], BF16, tag="ones_m")
            qdiag = [persist.tile([128, NB, 128], BF16, tag=f"qdiag{p}",
                                  name=f"qdiag{p}") for p in range(PAIRS)]
            kT = [persist.tile([128, S], BF16, tag=f"kT{p}", name=f"kT{p}")
                  for p in range(PAIRS)]
            # vp: kv-chunk layout [kv%128, chunk, (2h,HD)]; chunk c holds
            # blocks (2c, 2c+1) on partition halves (0:64, 64:128).
            vp = [persist.tile([128, NB // 2, 128], BF16, tag=f"vp{p}",
                               name=f"vp{p}") for p in range(PAIRS)]
            # vshift: shifted by one block; chunk c holds blocks (2c+1, 2c+2);
            # [0:64, 15] holds block 31, [64:128, 15] is unused garbage.
            vshift = [persist.tile([128, NB // 2, 128], BF16, tag=f"vsh{p}",
                                   name=f"vsh{p}") for p in range(PAIRS)]
            # vz1/vz2: top-half-zero copies so a lone high-half block can be
            # contracted with a full-128 matmul (accumulation groups cannot
            # mix the two 64-partition row halves, but full+half is legal).
            # vz1[64:128, c] = block 2c+1; vz2[64:128, c] = block 2c+2.
            vz1 = [persist.tile([128, NB // 2, 128], BF16, tag=f"vz1{p}",
                                name=f"vz1{p}") for p in range(PAIRS)]
            vz2 = [persist.tile([128, NB // 2, 128], BF16, tag=f"vz2{p}",
                                name=f"vz2{p}") for p in range(PAIRS)]
            attn = [persist.tile([128, NB, BS], BF16, tag=f"attn{p}",
                                 name=f"attn{p}") for p in range(PAIRS)]

            # ---- input loads (wq + first xT tile first, so PE starts ASAP) ----
            wq_view = wq_d.ap().rearrange("(c p) m -> p c m", p=128)
            xT_view = xT_d.ap().rearrange("(c p) s -> p c s", p=128)
            nc.sync.dma_start(wq_s[:, 0:4, :], wq_view[:, 0:4, :])
            nc.sync.dma_start(wq_s[:, 4:8, :], wq_view[:, 4:8, :])
            nc.sync.dma_start(xT_s[:, 0:4, 0:ST], xT_view[:, 0:4, 0:ST])
            nc.sync.dma_start(xT_s[:, 4:8, 0:ST], xT_view[:, 4:8, 0:ST])
            nc.gpsimd.dma_start(bq_s[:], bq_d.ap().rearrange("(pp p) -> p pp", p=128))
            nc.gpsimd.dma_start(bk_s[:], bk_d.ap().rearrange("(pp p) -> p pp", p=128))
            nc.gpsimd.dma_start(bv_s[:], bv_d.ap().rearrange("(pp p) -> p pp", p=128))
            nc.sync.dma_start(wk_s[:], wk_d.ap().rearrange("(c p) m -> p c m", p=128))
            nc.sync.dma_start(wv_s[:], wv_d.ap().rearrange("(c p) m -> p c m", p=128))
            for t in range(1, NST):
                sl = slice(t * ST, (t + 1) * ST)
                nc.sync.dma_start(xT_s[:, :, sl], xT_view[:, :, sl])
            # wo is consumed only by the (late) output projection
            nc.sync.dma_start(
                wo_s[:],
                wo_d.ap().rearrange("(pp p) (h f) -> p pp h f", p=128, f=512),
            )

            nc.gpsimd.memset(ones_m[:], 1.0)
            # qdiag off-diagonal quadrants must be zero; pair 0 is needed
            # almost immediately, so zero it on the (otherwise idle) DVE.
            nc.vector.memset(qdiag[0][:], 0.0)
            nc.gpsimd.memset(qdiag[1][:], 0.0)
            for p in range(PAIRS):
                nc.gpsimd.memset(vz1[p][0:64, :, :], 0.0)
                nc.vector.memset(vz2[p][0:64, :, :], 0.0)

            kTv = [kT[p][:].rearrange("p (b c) -> p b c", c=BS)
                   for p in range(PAIRS)]

            def v_block_ap(p, blk, half):
                """[64, 128] AP of V block `blk` at partition half `half`."""
                hs = slice(half * 64, half * 64 + 64)
                if blk % 2 == half:
                    return vp[p][hs, blk // 2, :]
                if half == 0:
                    assert blk % 2 == 1
                    return vshift[p][hs, (blk - 1) // 2, :]
                assert blk % 2 == 0 and blk >= 2
                return vshift[p][hs, blk // 2 - 1, :]

            # ---- global rows: full attention over all 16 kv chunks ----
            def glob_unit(p, i, sc_g, pv_g, sm_g, pt_g, ep_g):
                NCH = NB // 2  # 16 kv chunks, in two 8-chunk halves
                ps_pv = pv_g.tile([128, 128], F32, tag="pv", name="pv")
                ps_sm = sm_g.tile([128, 128], F32, tag="sm", name="sm")
                for half in range(2):
                    c0 = half * (NCH // 2)
                    ps_s = sc_g.tile([128, NCH // 2, 128], F32, tag="sc", name="sc")
                    PT = pt_g.tile([128, NCH // 2, 128], BF16, tag="pt", name="pt")
                    for cc in range(NCH // 2):
                        nc.tensor.matmul(
                            ps_s[:, cc, :],
                            kT[p][:, (c0 + cc) * 128:(c0 + cc + 1) * 128],
                            qdiag[p][:, i, :],
                            start=True, stop=True,
                        )
                    for c4 in range(2):   # one ACT per PSUM bank
                        sl = slice(c4 * 4, c4 * 4 + 4)
                        nc.scalar.activation(
                            PT[:, sl, :], ps_s[:, sl, :],
                            mybir.ActivationFunctionType.Exp, scale=SCALE,
                        )
                    for cc in range(NCH // 2):
                        nc.tensor.matmul(
                            ps_pv[:], vp[p][:, c0 + cc, :], PT[:, cc, :],
                            start=(half == 0 and cc == 0),
                            stop=(half == 1 and cc == NCH // 2 - 1),
                        )
                    for cc in range(NCH // 2):
                        nc.tensor.matmul(
                            ps_sm[:], ones_m[:], PT[:, cc, :],
                            start=(half == 0 and cc == 0),
                            stop=(half == 1 and cc == NCH // 2 - 1),
                        )
                rc = ep_g.tile([128, 128], F32, tag="rc", name="rc")
                nc.vector.reciprocal(rc[:], ps_sm[:])
                nc.vector.tensor_tensor(
                    attn[p][0:64, i, :], ps_pv[0:64, 0:64],
                    rc[0:64, 0:64], mybir.AluOpType.mult)
                nc.vector.tensor_tensor(
                    attn[p][64:128, i, :], ps_pv[64:128, 64:128],
                    rc[64:128, 64:128], mybir.AluOpType.mult)

            # ---- phase 1: Q/K/V projections (+ global rows per pair) ----
            with (
                tc.tile_pool(name="ps_proj", bufs=cfg["ps_proj"], space="PSUM") as ps_proj,
                tc.tile_pool(name="vt_tmp", bufs=3) as vt_pool,
                tc.tile_pool(name="sc_g", bufs=1, space="PSUM") as sc_g,
                tc.tile_pool(name="pv_g", bufs=1, space="PSUM") as pv_g,
                tc.tile_pool(name="sm_g", bufs=1, space="PSUM") as sm_g,
                tc.tile_pool(name="pt_g", bufs=2) as pt_g,
                tc.tile_pool(name="ep_g", bufs=2) as ep_g,
            ):
                for p in range(PAIRS):
                    pcol = slice(p * 128, (p + 1) * 128)
                    for t in range(NST):
                        ssl = slice(t * ST, (t + 1) * ST)
                        # Q -> qT -> qdiag (block-diagonal per q block)
                        ps = ps_proj.tile([128, ST], F32, tag="ps")
                        for c in range(EC):
                            nc.tensor.matmul(
                                ps[:], wq_s[:, c, pcol], xT_s[:, c, ssl],
                                start=(c == 0), stop=(c == EC - 1),
                            )
                        nblk = ST // BS  # 8 blocks per S tile
                        b0 = t * nblk
                        src = ps.rearrange("q (nb f) -> q nb f", f=BS)
                        nc.vector.tensor_scalar(
                            qdiag[p][0:64, b0:b0 + nblk, 0:64],
                            src[0:64], bq_s[0:64, p:p + 1], None,
                            mybir.AluOpType.add,
                        )
                        nc.vector.tensor_scalar(
                            qdiag[p][64:128, b0:b0 + nblk, 64:128],
                            src[64:128], bq_s[64:128, p:p + 1], None,
                            mybir.AluOpType.add,
                        )
                        # K -> kT
                        ps = ps_proj.tile([128, ST], F32, tag="ps")
                        for c in range(EC):
                            nc.tensor.matmul(
                                ps[:], wk_s[:, c, pcol], xT_s[:, c, ssl],
                                start=(c == 0), stop=(c == EC - 1),
                            )
                        nc.scalar.activation(
                            kT[p][:, ssl], ps[:],
                            mybir.ActivationFunctionType.Identity,
                            bias=bk_s[:, p:p + 1],
                        )
                        # V -> vT tmp -> DMA-transpose -> vp chunks
                        ps = ps_proj.tile([128, ST], F32, tag="ps")
                        for c in range(EC):
                            nc.tensor.matmul(
                                ps[:], wv_s[:, c, pcol], xT_s[:, c, ssl],
                                start=(c == 0), stop=(c == EC - 1),
                            )
                        vt = vt_pool.tile([128, ST], BF16, tag="vt")
                        nc.vector.tensor_scalar(
                            vt[:], ps[:], bv_s[:, p:p + 1], None,
                            mybir.AluOpType.add,
                        )
                        for j in range(ST // 128):
                            nc.sync.dma_start(
                                vp[p][:, t * (ST // 128) + j, :],
                                vt[:, j * 128:(j + 1) * 128],
                                transpose=True,
                            )
                    # vshift: chunk c = blocks (2c+1, 2c+2)
                    nc.gpsimd.dma_start(vshift[p][0:64, 0:16, :],
                                        vp[p][64:128, 0:16, :])
                    nc.gpsimd.dma_start(vshift[p][64:128, 0:15, :],
                                        vp[p][0:64, 1:16, :])
                    nc.gpsimd.dma_start(vz1[p][64:128, 0:16, :],
                                        vp[p][64:128, 0:16, :])
                    nc.gpsimd.dma_start(vz2[p][64:128, 0:15, :],
                                        vp[p][0:64, 1:16, :])
                    for i in glob_rows:
                        glob_unit(p, i, sc_g, pv_g, sm_g, pt_g, ep_g)

            # ---- phase 2: middle rows (6 kv blocks in 3 psum slots each) ----
            out_view = out_d.ap().rearrange("(g t p) e -> p g t e", p=128, t=2)
            done_rows = set(glob_rows)
            emitted_t = set()
            ncopy = 0
            with (
                tc.tile_pool(name="sc_m", bufs=cfg["sc_m"], space="PSUM") as sc_m,
                tc.tile_pool(name="pv_m", bufs=cfg["pv_m"], space="PSUM") as pv_m,
                tc.tile_pool(name="sm_m", bufs=cfg["sm_m"], space="PSUM") as sm_m,
                tc.tile_pool(name="ps_oi", bufs=1, space="PSUM") as ps_oi,
                tc.tile_pool(name="pt_m", bufs=cfg["pt_m"]) as pt_m,
                tc.tile_pool(name="ep_m", bufs=cfg["rc_m"]) as ep_m,
                tc.tile_pool(name="o_grp", bufs=2) as o_pool,
            ):
                o_cur = [None]

                def mid_front(p, i):
                    prs = mid_pairs[i]
                    ps_s = sc_m.tile([128, 3, 128], F32, tag="sc", name="sc")
                    for j, pr in enumerate(prs):
                        if pr[0] == 'c':
                            a = pr[1]
                            nc.tensor.matmul(
                                ps_s[:, j, :],
                                kT[p][:, a * BS:(a + 2) * BS],
                                qdiag[p][:, i, :],
                                start=True, stop=True,
                            )
                        else:
                            a, b = pr[1], pr[2]
                            nc.tensor.matmul(
                                ps_s[0:64, j, :], kTv[p][:, a, :],
                                qdiag[p][:, i, :], start=True, stop=True,
                            )
                            nc.tensor.matmul(
                                ps_s[64:128, j, :], kTv[p][:, b, :],
                                qdiag[p][:, i, :], start=True, stop=True,
                            )
                    PT = pt_m.tile([128, 3, 128], BF16, tag="pt", name="pt")
                    nc.scalar.activation(
                        PT[:], ps_s[:],
                        mybir.ActivationFunctionType.Exp, scale=SCALE)
                    return PT

                def mid_back(p, i, PT):
                    prs = mid_pairs[i]
                    ps_pv = pv_m.tile([128, 128], F32, tag="pv", name="pv")
                    ps_sm = sm_m.tile([128, 128], F32, tag="sm", name="sm")
                    if _env_flag("KERNEL_NO_PV"):
                        nc.tensor.matmul(ps_pv[:], vp[p][:, 0, :], PT[:, 0, :], start=True, stop=True)
                        for j in range(3):
                            nc.tensor.matmul(ps_sm[:], ones_m[:], PT[:, j, :], start=(j == 0), stop=(j == 2))
                        rcq = ep_m.tile([128, 128], F32, tag="rc", name="rcq")
                        nc.vector.reciprocal(rcq[:], ps_sm[:])
                        nc.vector.tensor_tensor(attn[p][0:64, i, :], ps_pv[0:64, 0:64], rcq[0:64, 0:64], mybir.AluOpType.mult)
                        nc.vector.tensor_tensor(attn[p][64:128, i, :], ps_pv[64:128, 64:128], rcq[64:128, 64:128], mybir.AluOpType.mult)
                        if p == PAIRS - 1:
                            done_rows.add(i)
                            if not _env_flag("KERNEL_NO_EMIT"):
                                emit_ready()
                        return
                    mms = []  # (lhsT, rhs) in full/lo-only order
                    los = []
                    for j, pr in enumerate(prs):
                        if pr[0] == 'c':
                            a = pr[1]
                            if a % 2 == 0:
                                lhs = vp[p][:, a // 2, :]
                            else:
                                lhs = vshift[p][:, (a - 1) // 2, :]
                            mms.append((lhs, PT[:, j, :]))
                        else:
                            a, b = pr[1], pr[2]
                            # b (high half) via top-zeroed full-128 weight
                            if b % 2 == 1:
                                zlhs = vz1[p][:, b // 2, :]
                            else:
                                zlhs = vz2[p][:, b // 2 - 1, :]
                            mms.append((zlhs, PT[:, j, :]))
                            los.append((v_block_ap(p, a, 0),
                                        PT[0:64, j, :]))
                    mms += los
                    for mi, (lhs, rhs) in enumerate(mms):
                        nc.tensor.matmul(
                            ps_pv[:], lhs, rhs,
                            start=(mi == 0), stop=(mi == len(mms) - 1),
                            skip_group_check=True)
                    for j in range(3):
                        nc.tensor.matmul(
                            ps_sm[:], ones_m[:], PT[:, j, :],
                            start=(j == 0), stop=(j == 2))
                    if _env_flag("KERNEL_NO_EPI"):
                        return
                    rc = ep_m.tile([128, 128], F32, tag="rc", name="rc")
                    nc.vector.reciprocal(rc[:], ps_sm[:])
                    nc.vector.tensor_tensor(
                        attn[p][0:64, i, :], ps_pv[0:64, 0:64],
                        rc[0:64, 0:64], mybir.AluOpType.mult)
                    nc.vector.tensor_tensor(
                        attn[p][64:128, i, :], ps_pv[64:128, 64:128],
                        rc[64:128, 64:128], mybir.AluOpType.mult)
                    if p == PAIRS - 1:
                        done_rows.add(i)
                        if not _env_flag("KERNEL_NO_EMIT"):
                            emit_ready()

                def emit_ready():
                    nonlocal ncopy
                    for t in range(S // 128):
                        if t in emitted_t:
                            continue
                        if 2 * t not in done_rows or 2 * t + 1 not in done_rows:
                            continue
                        emitted_t.add(t)
                        g, sub = t // 2, t % 2
                        if sub == 0:
                            o_cur[0] = o_pool.tile([128, 2, 1024], BF16,
                                                   tag="og", name="og")
                        og = o_cur[0]
                        pso = ps_oi.tile([128, 2, 512], F32, tag="po", name="po")
                        for h in range(2):
                            for pp in range(PAIRS):
                                nc.tensor.matmul(
                                    pso[:, h, :],
                                    attn[pp][:, 2 * t:2 * t + 2, :],
                                    wo_s[:, pp, h, :],
                                    start=(pp == 0), stop=(pp == PAIRS - 1),
                                )
                        if cfg["copy_acts"] and ncopy % 2 == 0:
                            nc.scalar.copy(og[:, sub, :], pso.rearrange("q h f -> q (h f)"))
                        else:
                            nc.vector.tensor_copy(og[:, sub, :],
                                                  pso.rearrange("q h f -> q (h f)"))
                        ncopy += 1
                        if sub == 1:
                            nc.gpsimd.dma_start(out_view[:, g, :, :], og[:])

                prev = None
                nmid = int(os.environ.get("KERNEL_NMID", "30"))
                for i in mid_rows[:nmid]:
                    for p in range(PAIRS):
                        PT = mid_front(p, i)
                        if prev is not None:
                            mid_back(*prev)
                        prev = (p, i, PT)
                if prev is not None:
                    mid_back(*prev)

    nc.compile()
    return nc


_cache = {}


def _get_program(block_mask, cfg=None):
    bm = np.asarray(block_mask)
    assert bm.shape == (S, S)
    blk = bm.reshape(NB, BS, NB, BS).any(axis=(1, 3))
    key = (blk.tobytes(), tuple(sorted((cfg or {}).items())))
    if key not in _cache:
        sel_lists = [list(np.nonzero(blk[i])[0]) for i in range(NB)]
        _cache[key] = (_build_program(sel_lists, cfg), sel_lists)
    return _cache[key]


def kernel(x, Wq, bq, Wk, bk, Wv, bv, Wo, bo, block_mask):
    global LAST_RESULTS
    x = np.asarray(x)
    nc, _ = _get_program(block_mask)

    bf = ml_dtypes.bfloat16
    in_maps = []
    for c in range(NCORES):
        b = c // GROUPS
        g = c % GROUPS
        cols = slice(g * COLS, (g + 1) * COLS)
        in_maps.append({
            "xT": np.ascontiguousarray(np.asarray(x)[b].T).astype(bf),
            "wq": np.ascontiguousarray(np.asarray(Wq)[:, cols]).astype(bf),
            "wk": np.ascontiguousarray(np.asarray(Wk)[:, cols]).astype(bf),
            "wv": np.ascontiguousarray(np.asarray(Wv)[:, cols]).astype(bf),
            "wo": np.ascontiguousarray(np.asarray(Wo)[cols, :]).astype(bf),
            "bq": np.ascontiguousarray(np.asarray(bq)[cols]).astype(np.float32),
            "bk": np.ascontiguousarray(np.asarray(bk)[cols]).astype(np.float32),
            "bv": np.ascontiguousarray(np.asarray(bv)[cols]).astype(np.float32),
        })

    trace = bool(int(os.environ.get("KERNEL_TRACE", "0")))
    try:
        res = run_bass_kernel_spmd(
            nc, in_maps, core_ids=list(range(NCORES)), trace=trace,
        )
    except ModuleNotFoundError:
        # axon NTFF profile hook not available in this container
        res = run_bass_kernel_spmd(
            nc, in_maps, core_ids=list(range(NCORES)), trace=False,
        )
    LAST_RESULTS = res

    out = np.zeros((B, S, E), dtype=np.float32)
    for c in range(NCORES):
        out[c // GROUPS] += res.results[c]["out"].astype(np.float32)
    out += np.asarray(bo, dtype=np.float32)
    return out


# revision 10
# speedup vs baseline: 1.2165x; 1.0568x over previous
"""BigBird block-sparse attention kernel for 8 Trainium2 NeuronCores.

Sharding: data-parallel over batch (B=2) x head-parallel over head groups
(16 heads -> 4 groups of 4). Core c handles batch c//4, heads [4*(c%4), 4*(c%4)+4).
Each core computes its Q/K/V projection column slice, block-sparse attention for
its 4 heads (processed as 2 "pairs" of 2 heads packed on 128 partitions), and a
partial output projection. Host sums the 4 partials per batch and adds bo.

Attention uses transposed score layout (scoresT[kv, q]) so probability tiles
feed the PV matmul directly. Middle rows pack their 6 kv blocks into 3
score-PSUM slots of 128 kv each; consecutive block pairs use a single
128-contraction matmul (kT columns are contiguous), arbitrary pairs fall back
to two 64-partition matmuls per slot. Softmax row sums always contract 128 kv
per matmul (3 ones-matmuls per row). PV uses vp (kv%128-major V) plus vshift
(V shifted by one block) so consecutive pairs contract 128 kv in one matmul.

Self-contained: hardcodes shapes; derives the block-sparsity structure from the
block_mask input at trace time.
"""

import os
import numpy as np
import ml_dtypes

import concourse.bass as bass
import concourse.mybir as mybir
import concourse.tile as tile
from concourse import bacc
from concourse.bass_utils import run_bass_kernel_spmd

F32 = mybir.dt.float32
BF16 = mybir.dt.bfloat16

B, S, E, H = 2, 2048, 1024, 16
BS = 64                      # block size
NB = S // BS                 # 32 blocks
HD = E // H                  # 64 head dim
SCALE = HD ** -0.5           # 0.125
NCORES = 8
GROUPS = 4                   # head groups (one per core within a batch)
COLS = E // GROUPS           # 256 projection cols per core
PAIRS = 2                    # head pairs per core (2 heads = 128 cols each)
EC = E // 128                # 8 contraction chunks
ST = 512                     # S tile for projections
NST = S // ST                # 4

LAST_RESULTS = None          # BassKernelResults of the last run (for test.py)

DEFAULT_CFG = dict(
    ps_proj=3, sc_m=3, pv_m=3, pt_m=4, rc_m=3, depth=2,
    copy_acts=True,          # outproj copies on ACT (else DVE)
)


def _env_flag(name, default="0"):
    return bool(int(os.environ.get(name, default)))


def _make_pairs(sel):
    """Pair the 6 sorted kv blocks of a middle row into 3 slots.

    Returns list of ('c', a) for a consecutive pair (a, a+1) or
    ('s', a, b) for an arbitrary pair. Greedy left-to-right consecutive
    matching; leftovers paired in order (a < b so b is never block 0).
    """
    used = [False] * len(sel)
    pairs = []
    if _env_flag("KERNEL_ALL_SPLIT"):
        return [('s', sel[0], sel[1]), ('s', sel[2], sel[3]), ('s', sel[4], sel[5])]
    i = 0
    while i < len(sel) - 1:
        if not used[i] and not used[i + 1] and sel[i + 1] == sel[i] + 1:
            pairs.append(('c', sel[i]))
            used[i] = used[i + 1] = True
            i += 2
        else:
            i += 1
    rest = [sel[i] for i in range(len(sel)) if not used[i]]
    for j in range(0, len(rest), 2):
        pairs.append(('s', rest[j], rest[j + 1]))
    assert len(pairs) == 3
    return pairs


def _build_program(sel_lists, cfg=None):
    """Build the SPMD bass program. sel_lists[i] = sorted kv block list of q block i."""
    cfg = dict(DEFAULT_CFG, **(cfg or {}))
    nc = bacc.Bacc("TRN2", target_bir_lowering=False, debug=False)

    xT_d = nc.dram_tensor("xT", [E, S], BF16, kind="ExternalInput")
    wq_d = nc.dram_tensor("wq", [E, COLS], BF16, kind="ExternalInput")
    wk_d = nc.dram_tensor("wk", [E, COLS], BF16, kind="ExternalInput")
    wv_d = nc.dram_tensor("wv", [E, COLS], BF16, kind="ExternalInput")
    wo_d = nc.dram_tensor("wo", [COLS, E], BF16, kind="ExternalInput")
    bq_d = nc.dram_tensor("bq", [COLS], F32, kind="ExternalInput")
    bk_d = nc.dram_tensor("bk", [COLS], F32, kind="ExternalInput")
    bv_d = nc.dram_tensor("bv", [COLS], F32, kind="ExternalInput")
    out_d = nc.dram_tensor("out", [S, E], BF16, kind="ExternalOutput")

    glob_rows = [i for i in range(NB) if len(sel_lists[i]) == NB]
    mid_rows = [i for i in range(NB) if len(sel_lists[i]) != NB]
    for i in mid_rows:
        assert len(sel_lists[i]) == 6, (i, len(sel_lists[i]))
    mid_pairs = {i: _make_pairs(sel_lists[i]) for i in mid_rows}

    with tile.TileContext(nc) as tc:
        with (
            tc.tile_pool(name="persist", bufs=1) as persist,
        ):
            # ---- persistent SBUF tensors ----
            xT_s = persist.tile([128, EC, S], BF16, tag="xT_s")
            wq_s = persist.tile([128, EC, COLS], BF16, tag="wq_s")
            wk_s = persist.tile([128, EC, COLS], BF16, tag="wk_s")
            wv_s = persist.tile([128, EC, COLS], BF16, tag="wv_s")
            wo_s = persist.tile([128, PAIRS, 2, 512], BF16, tag="wo_s")
            bq_s = persist.tile([128, PAIRS], F32, tag="bq_s")
            bk_s = persist.tile([128, PAIRS], F32, tag="bk_s")
            bv_s = persist.tile([128, PAIRS], F32, tag="bv_s")
            ones_m = persist.tile([128, 128], BF16, tag="ones_m")
            qdiag = [persist.tile([128, NB, 128], BF16, tag=f"qdiag{p}",
                                  name=f"qdiag{p}") for p in range(PAIRS)]
            kT = [persist.tile([128, S], BF16, tag=f"kT{p}", name=f"kT{p}")
                  for p in range(PAIRS)]
            # vp: kv-chunk layout [kv%128, chunk, (2h,HD)]; chunk c holds
            # blocks (2c, 2c+1) on partition halves (0:64, 64:128).
            vp = [persist.tile([128, NB // 2, 128], BF16, tag=f"vp{p}",
                               name=f"vp{p}") for p in range(PAIRS)]
            # vshift: shifted by one block; chunk c holds blocks (2c+1, 2c+2);
            # [0:64, 15] holds block 31, [64:128, 15] is unused garbage.
            vshift = [persist.tile([128, NB // 2, 128], BF16, tag=f"vsh{p}",
                                   name=f"vsh{p}") for p in range(PAIRS)]
            # vz1/vz2: top-half-zero copies so a lone high-half block can be
            # contracted with a full-128 matmul (accumulation groups cannot
            # mix the two 64-partition row halves, but full+half is legal).
            # vz1[64:128, c] = block 2c+1; vz2[64:128, c] = block 2c+2.
            vz1 = [persist.tile([128, NB // 2, 128], BF16, tag=f"vz1{p}",
                                name=f"vz1{p}") for p in range(PAIRS)]
            vz2 = [persist.tile([128, NB // 2, 128], BF16, tag=f"vz2{p}",
                                name=f"vz2{p}") for p in range(PAIRS)]
            attn = [persist.tile([128, NB, BS], BF16, tag=f"attn{p}",
                                 name=f"attn{p}") for p in range(PAIRS)]

            # ---- input loads (wq + first xT tile first, so PE starts ASAP) ----
            wq_view = wq_d.ap().rearrange("(c p) m -> p c m", p=128)
            xT_view = xT_d.ap().rearrange("(c p) s -> p c s", p=128)
            nc.sync.dma_start(wq_s[:, 0:4, :], wq_view[:, 0:4, :])
            nc.sync.dma_start(wq_s[:, 4:8, :], wq_view[:, 4:8, :])
            nc.sync.dma_start(xT_s[:, 0:4, 0:ST], xT_view[:, 0:4, 0:ST])
            nc.sync.dma_start(xT_s[:, 4:8, 0:ST], xT_view[:, 4:8, 0:ST])
            nc.gpsimd.dma_start(bq_s[:], bq_d.ap().rearrange("(pp p) -> p pp", p=128))
            nc.gpsimd.dma_start(bk_s[:], bk_d.ap().rearrange("(pp p) -> p pp", p=128))
            nc.gpsimd.dma_start(bv_s[:], bv_d.ap().rearrange("(pp p) -> p pp", p=128))
            nc.sync.dma_start(wk_s[:], wk_d.ap().rearrange("(c p) m -> p c m", p=128))
            nc.sync.dma_start(wv_s[:], wv_d.ap().rearrange("(c p) m -> p c m", p=128))
            for t in range(1, NST):
                sl = slice(t * ST, (t + 1) * ST)
                nc.sync.dma_start(xT_s[:, :, sl], xT_view[:, :, sl])
            # wo is consumed only by the (late) output projection
            nc.sync.dma_start(
                wo_s[:],
                wo_d.ap().rearrange("(pp p) (h f) -> p pp h f", p=128, f=512),
            )

            nc.gpsimd.memset(ones_m[:], 1.0)
            # qdiag off-diagonal quadrants must be zero; pair 0 is needed
            # almost immediately, so zero it on the (otherwise idle) DVE.
            nc.vector.memset(qdiag[0][:], 0.0)
            nc.gpsimd.memset(qdiag[1][:], 0.0)
            for p in range(PAIRS):
                nc.gpsimd.memset(vz1[p][0:64, :, :], 0.0)
                nc.vector.memset(vz2[p][0:64, :, :], 0.0)

            kTv = [kT[p][:].rearrange("p (b c) -> p b c", c=BS)
                   for p in range(PAIRS)]

            def v_block_ap(p, blk, half):
                """[64, 128] AP of V block `blk` at partition half `half`."""
                hs = slice(half * 64, half * 64 + 64)
                if blk % 2 == half:
                    return vp[p][hs, blk // 2, :]
                if half == 0:
                    assert blk % 2 == 1
                    return vshift[p][hs, (blk - 1) // 2, :]
                assert blk % 2 == 0 and blk >= 2
                return vshift[p][hs, blk // 2 - 1, :]

            # ---- global rows: full attention over all 16 kv chunks ----
            def glob_unit(p, i, sc_g, pv_g, sm_g, pt_g, ep_g):
                NCH = NB // 2  # 16 kv chunks, in two 8-chunk halves
                ps_pv = pv_g.tile([128, 128], F32, tag="pv", name="pv")
                ps_sm = sm_g.tile([128, 128], F32, tag="sm", name="sm")
                for half in range(2):
                    c0 = half * (NCH // 2)
                    ps_s = sc_g.tile([128, NCH // 2, 128], F32, tag="sc", name="sc")
                    PT = pt_g.tile([128, NCH // 2, 128], BF16, tag="pt", name="pt")
                    for cc in range(NCH // 2):
                        nc.tensor.matmul(
                            ps_s[:, cc, :],
                            kT[p][:, (c0 + cc) * 128:(c0 + cc + 1) * 128],
                            qdiag[p][:, i, :],
                            start=True, stop=True,
                        )
                    for c4 in range(2):   # one ACT per PSUM bank
                        sl = slice(c4 * 4, c4 * 4 + 4)
                        nc.scalar.activation(
                            PT[:, sl, :], ps_s[:, sl, :],
                            mybir.ActivationFunctionType.Exp, scale=SCALE,
                        )
                    for cc in range(NCH // 2):
                        nc.tensor.matmul(
                            ps_pv[:], vp[p][:, c0 + cc, :], PT[:, cc, :],
                            start=(half == 0 and cc == 0),
                            stop=(half == 1 and cc == NCH // 2 - 1),
                        )
                    for cc in range(NCH // 2):
                        nc.tensor.matmul(
                            ps_sm[:], ones_m[:], PT[:, cc, :],
                            start=(half == 0 and cc == 0),
                            stop=(half == 1 and cc == NCH // 2 - 1),
                        )
                rc = ep_g.tile([128, 128], F32, tag="rc", name="rc")
                nc.vector.reciprocal(rc[:], ps_sm[:])
                nc.vector.tensor_tensor(
                    attn[p][0:64, i, :], ps_pv[0:64, 0:64],
                    rc[0:64, 0:64], mybir.AluOpType.mult)
                nc.vector.tensor_tensor(
                    attn[p][64:128, i, :], ps_pv[64:128, 64:128],
                    rc[64:128, 64:128], mybir.AluOpType.mult)

            # ---- phase 1: Q/K/V projections (+ global rows per pair) ----
            with (
                tc.tile_pool(name="ps_proj", bufs=cfg["ps_proj"], space="PSUM") as ps_proj,
                tc.tile_pool(name="vt_tmp", bufs=3) as vt_pool,
                tc.tile_pool(name="sc_g", bufs=1, space="PSUM") as sc_g,
                tc.tile_pool(name="pv_g", bufs=1, space="PSUM") as pv_g,
                tc.tile_pool(name="sm_g", bufs=1, space="PSUM") as sm_g,
                tc.tile_pool(name="pt_g", bufs=2) as pt_g,
                tc.tile_pool(name="ep_g", bufs=2) as ep_g,
            ):
                for p in range(PAIRS):
                    pcol = slice(p * 128, (p + 1) * 128)
                    for t in range(NST):
                        ssl = slice(t * ST, (t + 1) * ST)
                        # Q -> qT -> qdiag (block-diagonal per q block)
                        ps = ps_proj.tile([128, ST], F32, tag="ps")
                        for c in range(EC):
                            nc.tensor.matmul(
                                ps[:], wq_s[:, c, pcol], xT_s[:, c, ssl],
                                start=(c == 0), stop=(c == EC - 1),
                            )
                        nblk = ST // BS  # 8 blocks per S tile
                        b0 = t * nblk
                        src = ps.rearrange("q (nb f) -> q nb f", f=BS)
                        nc.vector.tensor_scalar(
                            qdiag[p][0:64, b0:b0 + nblk, 0:64],
                            src[0:64], bq_s[0:64, p:p + 1], None,
                            mybir.AluOpType.add,
                        )
                        nc.vector.tensor_scalar(
                            qdiag[p][64:128, b0:b0 + nblk, 64:128],
                            src[64:128], bq_s[64:128, p:p + 1], None,
                            mybir.AluOpType.add,
                        )
                        # K -> kT
                        ps = ps_proj.tile([128, ST], F32, tag="ps")
                        for c in range(EC):
                            nc.tensor.matmul(
                                ps[:], wk_s[:, c, pcol], xT_s[:, c, ssl],
                                start=(c == 0), stop=(c == EC - 1),
                            )
                        nc.scalar.activation(
                            kT[p][:, ssl], ps[:],
                            mybir.ActivationFunctionType.Identity,
                            bias=bk_s[:, p:p + 1],
                        )
                        # V -> vT tmp -> DMA-transpose -> vp chunks
                        ps = ps_proj.tile([128, ST], F32, tag="ps")
                        for c in range(EC):
                            nc.tensor.matmul(
                                ps[:], wv_s[:, c, pcol], xT_s[:, c, ssl],
                                start=(c == 0), stop=(c == EC - 1),
                            )
                        vt = vt_pool.tile([128, ST], BF16, tag="vt")
                        nc.vector.tensor_scalar(
                            vt[:], ps[:], bv_s[:, p:p + 1], None,
                            mybir.AluOpType.add,
                        )
                        for j in range(ST // 128):
                            nc.sync.dma_start(
                                vp[p][:, t * (ST // 128) + j, :],
                                vt[:, j * 128:(j + 1) * 128],
                                transpose=True,
                            )
                    # vshift: chunk c = blocks (2c+1, 2c+2)
                    nc.gpsimd.dma_start(vshift[p][0:64, 0:16, :],
                                        vp[p][64:128, 0:16, :])
                    nc.gpsimd.dma_start(vshift[p][64:128, 0:15, :],
                                        vp[p][0:64, 1:16, :])
                    nc.gpsimd.dma_start(vz1[p][64:128, 0:16, :],
                                        vp[p][64:128, 0:16, :])
                    nc.gpsimd.dma_start(vz2[p][64:128, 0:15, :],
                                        vp[p][0:64, 1:16, :])
                    for i in glob_rows:
                        glob_unit(p, i, sc_g, pv_g, sm_g, pt_g, ep_g)

            # ---- phase 2: middle rows (6 kv blocks in 3 psum slots each) ----
            out_view = out_d.ap().rearrange("(g t p) e -> p g t e", p=128, t=2)
            done_rows = set(glob_rows)
            emitted_t = set()
            ncopy = 0
            with (
                tc.tile_pool(name="sc_m", bufs=cfg["sc_m"], space="PSUM") as sc_m,
                tc.tile_pool(name="pv_m", bufs=cfg["pv_m"], space="PSUM") as pv_m,
                tc.tile_pool(name="ps_oi", bufs=1, space="PSUM") as ps_oi,
                tc.tile_pool(name="pt_m", bufs=cfg["pt_m"]) as pt_m,
                tc.tile_pool(name="ep_m", bufs=cfg["rc_m"]) as ep_m,
                tc.tile_pool(name="o_grp", bufs=2) as o_pool,
            ):
                o_cur = [None]

                def mid_front(p, i):
                    prs = mid_pairs[i]
                    ps_s = sc_m.tile([128, 3, 128], F32, tag="sc", name="sc")
                    for j, pr in enumerate(prs):
                        if pr[0] == 'c':
                            a = pr[1]
                            nc.tensor.matmul(
                                ps_s[:, j, :],
                                kT[p][:, a * BS:(a + 2) * BS],
                                qdiag[p][:, i, :],
                                start=True, stop=True,
                            )
                        else:
                            a, b = pr[1], pr[2]
                            nc.tensor.matmul(
                                ps_s[0:64, j, :], kTv[p][:, a, :],
                                qdiag[p][:, i, :], start=True, stop=True,
                            )
                            nc.tensor.matmul(
                                ps_s[64:128, j, :], kTv[p][:, b, :],
                                qdiag[p][:, i, :], start=True, stop=True,
                            )
                    PT = pt_m.tile([128, 3, 128], BF16, tag="pt", name="pt")
                    nc.scalar.activation(
                        PT[:], ps_s[:],
                        mybir.ActivationFunctionType.Exp, scale=SCALE)
                    return PT

                def mid_back(p, i, PT):
                    prs = mid_pairs[i]
                    pvsm = pv_m.tile([128, 2, 128], F32, tag="pv", name="pvsm")
                    ps_pv = pvsm[:, 0, :]
                    ps_sm = pvsm[:, 1, :]
                    if _env_flag("KERNEL_NO_PV"):
                        nc.tensor.matmul(ps_pv[:], vp[p][:, 0, :], PT[:, 0, :], start=True, stop=True)
                        for j in range(3):
                            nc.tensor.matmul(ps_sm, ones_m[:], PT[:, j, :], start=(j == 0), stop=(j == 2))
                        rcq = ep_m.tile([128, 128], F32, tag="rc", name="rcq")
                        nc.vector.reciprocal(rcq[:], ps_sm)
                        nc.vector.tensor_tensor(attn[p][0:64, i, :], pvsm[0:64, 0, 0:64], rcq[0:64, 0:64], mybir.AluOpType.mult)
                        nc.vector.tensor_tensor(attn[p][64:128, i, :], pvsm[64:128, 0, 64:128], rcq[64:128, 64:128], mybir.AluOpType.mult)
                        if p == PAIRS - 1:
                            done_rows.add(i)
                            if not _env_flag("KERNEL_NO_EMIT"):
                                emit_ready()
                        return
                    mms = []  # (lhsT, rhs) in full/lo-only order
                    los = []
                    for j, pr in enumerate(prs):
                        if pr[0] == 'c':
                            a = pr[1]
                            if a % 2 == 0:
                                lhs = vp[p][:, a // 2, :]
                            else:
                                lhs = vshift[p][:, (a - 1) // 2, :]
                            mms.append((lhs, PT[:, j, :]))
                        else:
                            a, b = pr[1], pr[2]
                            # b (high half) via top-zeroed full-128 weight
                            if b % 2 == 1:
                                zlhs = vz1[p][:, b // 2, :]
                            else:
                                zlhs = vz2[p][:, b // 2 - 1, :]
                            mms.append((zlhs, PT[:, j, :]))
                            los.append((v_block_ap(p, a, 0),
                                        PT[0:64, j, :]))
                    mms += los
                    for mi, (lhs, rhs) in enumerate(mms):
                        nc.tensor.matmul(
                            ps_pv, lhs, rhs,
                            start=(mi == 0), stop=(mi == len(mms) - 1),
                            skip_group_check=True)
                    for j in range(3):
                        nc.tensor.matmul(
                            ps_sm[:], ones_m[:], PT[:, j, :],
                            start=(j == 0), stop=(j == 2))
                    if _env_flag("KERNEL_NO_EPI"):
                        return
                    rc = ep_m.tile([128, 128], F32, tag="rc", name="rc")
                    nc.vector.reciprocal(rc[:], ps_sm[:])
                    nc.vector.tensor_tensor(
                        attn[p][0:64, i, :], pvsm[0:64, 0, 0:64],
                        rc[0:64, 0:64], mybir.AluOpType.mult)
                    nc.vector.tensor_tensor(
                        attn[p][64:128, i, :], pvsm[64:128, 0, 64:128],
                        rc[64:128, 64:128], mybir.AluOpType.mult)
                    if p == PAIRS - 1:
                        done_rows.add(i)
                        if not _env_flag("KERNEL_NO_EMIT"):
                            emit_ready()

                def emit_ready():
                    nonlocal ncopy
                    for t in range(S // 128):
                        if t in emitted_t:
                            continue
                        if 2 * t not in done_rows or 2 * t + 1 not in done_rows:
                            continue
                        emitted_t.add(t)
                        g, sub = t // 2, t % 2
                        if sub == 0:
                            o_cur[0] = o_pool.tile([128, 2, 1024], BF16,
                                                   tag="og", name="og")
                        og = o_cur[0]
                        pso = ps_oi.tile([128, 2, 512], F32, tag="po", name="po")
                        for h in range(2):
                            for pp in range(PAIRS):
                                nc.tensor.matmul(
                                    pso[:, h, :],
                                    attn[pp][:, 2 * t:2 * t + 2, :],
                                    wo_s[:, pp, h, :],
                                    start=(pp == 0), stop=(pp == PAIRS - 1),
                                )
                        if cfg["copy_acts"]:
                            nc.scalar.copy(og[:, sub, :], pso.rearrange("q h f -> q (h f)"))
                        else:
                            nc.vector.tensor_copy(og[:, sub, :],
                                                  pso.rearrange("q h f -> q (h f)"))
                        ncopy += 1
                        if sub == 1:
                            nc.gpsimd.dma_start(out_view[:, g, :, :], og[:])

                from collections import deque
                depth = cfg["depth"]
                pend = deque()
                nmid = int(os.environ.get("KERNEL_NMID", "30"))
                for i in mid_rows[:nmid]:
                    for p in range(PAIRS):
                        PT = mid_front(p, i)
                        pend.append((p, i, PT))
                        if len(pend) > depth:
                            mid_back(*pend.popleft())
                while pend:
                    mid_back(*pend.popleft())

    nc.compile()
    return nc


_cache = {}


def _get_program(block_mask, cfg=None):
    bm = np.asarray(block_mask)
    assert bm.shape == (S, S)
    blk = bm.reshape(NB, BS, NB, BS).any(axis=(1, 3))
    key = (blk.tobytes(), tuple(sorted((cfg or {}).items())))
    if key not in _cache:
        sel_lists = [list(np.nonzero(blk[i])[0]) for i in range(NB)]
        _cache[key] = (_build_program(sel_lists, cfg), sel_lists)
    return _cache[key]


def kernel(x, Wq, bq, Wk, bk, Wv, bv, Wo, bo, block_mask):
    global LAST_RESULTS
    x = np.asarray(x)
    nc, _ = _get_program(block_mask)

    bf = ml_dtypes.bfloat16
    in_maps = []
    for c in range(NCORES):
        b = c // GROUPS
        g = c % GROUPS
        cols = slice(g * COLS, (g + 1) * COLS)
        in_maps.append({
            "xT": np.ascontiguousarray(np.asarray(x)[b].T).astype(bf),
            "wq": np.ascontiguousarray(np.asarray(Wq)[:, cols]).astype(bf),
            "wk": np.ascontiguousarray(np.asarray(Wk)[:, cols]).astype(bf),
            "wv": np.ascontiguousarray(np.asarray(Wv)[:, cols]).astype(bf),
            "wo": np.ascontiguousarray(np.asarray(Wo)[cols, :]).astype(bf),
            "bq": np.ascontiguousarray(np.asarray(bq)[cols]).astype(np.float32),
            "bk": np.ascontiguousarray(np.asarray(bk)[cols]).astype(np.float32),
            "bv": np.ascontiguousarray(np.asarray(bv)[cols]).astype(np.float32),
        })

    trace = bool(int(os.environ.get("KERNEL_TRACE", "0")))
    try:
        res = run_bass_kernel_spmd(
            nc, in_maps, core_ids=list(range(NCORES)), trace=trace,
        )
    except ModuleNotFoundError:
        # axon NTFF profile hook not available in this container
        res = run_bass_kernel_spmd(
            nc, in_maps, core_ids=list(range(NCORES)), trace=False,
        )
    LAST_RESULTS = res

    out = np.zeros((B, S, E), dtype=np.float32)
    for c in range(NCORES):
        out[c // GROUPS] += res.results[c]["out"].astype(np.float32)
    out += np.asarray(bo, dtype=np.float32)
    return out
